# revision 35
# baseline (speedup 1.0000x reference)
"""Trainium2 Bass kernel for the low-rank linear operator.

Math: the reference collapses algebraically. With y = linspace(-1,1,H),
x = linspace(-1,1,W), dx = 2/(W-1):

  Vy[b,i] = sum_{h,w} v[b,i,h,w] * y_h
  Vx[b,i] = sum_{h,w} v[b,i,h,w] * x_w
  inner[b,r] = dx * sum_i (Vy[b,i]*psi[r,i,0] + Vx[b,i]*psi[r,i,1])
  A[b,o] = sum_r inner[b,r]*phi[o,r,0];  Bc[b,o] = sum_r inner[b,r]*phi[o,r,1]
  u[b,o,h,w] = A[b,o]*y_h + Bc[b,o]*x_w

Sharding: data-parallel over batch, 2 batches per core, 8 cores, no
collectives.

The problem is HBM-bandwidth bound (read v, write u); bf16 streams halve
DMA traffic vs f32 (measured rel err ~4e-4 against the 2e-2 gate). Host
pre-transposes v to [b, p=h//2, i, hh=h%2, w] bf16 so each input DMA
descriptor is 16KB contiguous; u is produced in the mirrored layout (8KB
descriptors) and re-transposed on the host.

Reduction: partition p = h//2. For each channel ch one matmul with a
sliding-window lhsT (zeros except col 2ch -> y_even values, col 2ch+1 ->
ones) over rhs [128, (hh,w)=512] accumulates, for ALL 64 channels, the
y_even-weighted row sums (psum row 2ch) and plain column sums (row
2ch+1) into a single [128, 512] f32 psum bank. Full-width DVE
mult+reduce passes against wty (1; dy on the hh=1 half) and wtx (0; x)
then give the gy/gx vectors feeding tiny f32 matmuls:
inner -> (A,B) -> per-partition scale/bias tiles. DVE/ACT/Pool
tensor_scalar ops generate u tiles as x_w*B + (y_even|y_odd)*A.

All constants ride in two packed tensors (one f32, one bf16) loaded on
the scalar DMA ring so the first v read issues immediately on sync.
"""

import sys

try:
    import concourse.bass as bass  # noqa: F401
except ImportError:
    for _p in ("/opt/trn_rl_repo", "/root/.axon_site/_ro/trn_rl_repo"):
        if _p not in sys.path:
            sys.path.insert(0, _p)

import numpy as np

import concourse.bacc as bacc
import concourse.bass as bass
import concourse.mybir as mybir
import concourse.tile as tile
from concourse.bass_utils import run_bass_kernel_spmd

F32 = mybir.dt.float32
BF16 = mybir.dt.bfloat16
MULT = mybir.AluOpType.mult
ADD = mybir.AluOpType.add

B, CI, CO, R, H, W = 16, 64, 64, 64, 256, 256
N_CORES = 8
BPC = B // N_CORES  # batches per core
HP = H // 2         # h-pairs per partition dim

IBLK = 16           # input channels per DMA (2MB bf16, 16KB descriptors)
NIB = CI // IBLK
OBLK = 8            # output channels per DMA (1MB bf16, 8KB descriptors)
NOB = CO // OBLK

# packed-constant column offsets (cf32 [128, CF32_W] f32)
_WTY = 0            # [128, 512]
_WTX = 512          # [128, 512]
_PSIY = 1024        # [128, 64]
_PSIX = 1088        # [128, 64]
_PHI = 1152         # [64, 128]
_YBC = 1280         # [1, 384]: y_even | y_odd | ones
_ID1 = 1664         # [1, 1]
CF32_W = 1665
# cf16 [128, CBF16_W] bf16: sliding-window lhsT table then x replicated
_YLHS = 0           # [128, 384]: col 128 = y_even, col 129 = ones
_XREP = 384         # [128, 256]
CBF16_W = 640

# generation-engine rotation, rates ~ DVE 354ns / ACT 585ns / Pool 655ns
_GEN_ENGINES = (
    "dve", "act", "pool", "dve", "dve", "act", "pool", "dve",
    "act", "pool", "dve", "dve", "act", "pool", "dve",
)


def build_nc():
    nc = bacc.Bacc("TRN2", target_bir_lowering=False, debug=False)

    v5 = nc.dram_tensor("v5", [BPC, HP, CI, 2, W], BF16, kind="ExternalInput")
    cf32d = nc.dram_tensor("cf32", [128, CF32_W], F32, kind="ExternalInput")
    cf16d = nc.dram_tensor("cf16", [128, CBF16_W], BF16, kind="ExternalInput")
    u5 = nc.dram_tensor("u5", [BPC, HP, CO, 2, W], BF16, kind="ExternalOutput")

    with tile.TileContext(nc) as tc:
        with (
            tc.tile_pool(name="consts", bufs=1) as consts,
            tc.tile_pool(name="inp", bufs=4) as in_pool,
            tc.tile_pool(name="outp", bufs=5) as out_pool,
            tc.tile_pool(name="scr", bufs=3) as scratch,
            tc.tile_pool(name="bc", bufs=6) as bc_pool,
            tc.tile_pool(name="psumA", bufs=2, space="PSUM") as psum_a,
            tc.tile_pool(name="psumT", bufs=1, space="PSUM") as psum_t,
            tc.tile_pool(name="psumBC", bufs=3, space="PSUM") as psum_bc,
        ):
            # cf16 gates the first matmul: tiny, lands first on the sync ring
            # ahead of the v reads. cf32 is only needed by stage-2 (~30us in);
            # the gpsimd ring keeps it off both streaming rings.
            cf16 = consts.tile([128, CBF16_W], BF16)
            nc.sync.dma_start(cf16[:], cf16d[:])
            cf32 = consts.tile([128, CF32_W], F32)
            nc.scalar.dma_start(cf32[:], cf32d[:])

            wty = cf32[:, _WTY : _WTY + 2 * W]
            wtx = cf32[:, _WTX : _WTX + 2 * W]
            psi2y = cf32[:, _PSIY : _PSIY + R]
            psi2x = cf32[:, _PSIX : _PSIX + R]
            phicat = cf32[0:R, _PHI : _PHI + 2 * CO]
            ybc = cf32[0:1, _YBC : _YBC + 384]
            id1 = cf32[0:1, _ID1 : _ID1 + 1]
            xrep = cf16[:, _XREP : _XREP + W]

            # HAM pre-warm: wait-free dummy matmuls spanning the whole
            # dead window [~6.6us preamble-end .. ~16.4us first-chunk-lands]
            # keep the PE clock at 8/8 so the real reduction matmuls track
            # the read stream at warm cadence. (A shorter dummy burst cools
            # again: any >3.4us idle window drops the clock to 4/8.)
            wsb = consts.tile([128, 64], BF16)
            nc.vector.memset(wsb[:], 0.0)
            wps = psum_t.tile([2, 64], F32, tag="warm")
            for _k in range(176):
                nc.tensor.matmul(
                    wps[:], lhsT=wsb[:, 0:2], rhs=wsb[:],
                    start=(_k == 0), stop=(_k == 175),
                )
            wrd = scratch.tile([2, 64], F32, tag="wrd")
            nc.vector.tensor_copy(wrd[:], wps[:])

            # per-batch reduction vectors, one column per batch
            gy_sb = consts.tile([2 * CI, BPC], F32)
            gx_sb = consts.tile([2 * CI, BPC], F32)

            def stage_a(b, interleave=None):
                """Reduce v[b] -> gy_sb/gx_sb[:, b]."""
                inter = interleave() if interleave is not None else None
                ps = psum_a.tile([128, 2, W], F32, tag="A")
                for blk in range(NIB):
                    t = in_pool.tile([128, IBLK, 2, W], BF16, tag="in")
                    nc.sync.dma_start(
                        t[:],
                        v5[b, :, blk * IBLK : (blk + 1) * IBLK, :, :],
                    )
                    for ii in range(IBLK):
                        ch = blk * IBLK + ii
                        lo = _YLHS + 128 - 2 * ch
                        nc.tensor.matmul(
                            ps[:],
                            lhsT=cf16[:, lo : lo + 128],
                            rhs=t[:, ii, :, :],
                            start=(ch == 0),
                            stop=(ch == CI - 1),
                        )
                    if inter is not None:
                        next(inter, None)
                        next(inter, None)
                psv = ps[:].rearrange("p hh w -> p (hh w)")
                sc = scratch.tile([128, 2 * W], F32, tag="sc")
                nc.vector.tensor_tensor(out=sc[:], in0=psv, in1=wty, op=MULT)
                nc.vector.tensor_reduce(
                    out=gy_sb[:, b : b + 1], in_=sc[:],
                    axis=mybir.AxisListType.X, op=ADD,
                )
                sc2 = scratch.tile([128, 2 * W], F32, tag="sc")
                nc.vector.tensor_tensor(out=sc2[:], in0=psv, in1=wtx, op=MULT)
                nc.vector.tensor_reduce(
                    out=gx_sb[:, b : b + 1], in_=sc2[:],
                    axis=mybir.AxisListType.X, op=ADD,
                )

            def tiny(b, out):
                """gy/gx[:, b] -> scale/bias tiles; emitted in 3 pumps so
                each PE hop's DVE-side input exists before the PE reaches it
                (no PE-queue stall between batch-1 matmul chunks)."""
                innert_ps = psum_t.tile([R, 1], F32, tag="tiny")
                nc.tensor.matmul(
                    innert_ps[:], lhsT=psi2y, rhs=gy_sb[:, b : b + 1],
                    start=True, stop=False,
                )
                nc.tensor.matmul(
                    innert_ps[:], lhsT=psi2x, rhs=gx_sb[:, b : b + 1],
                    start=False, stop=True,
                )
                sb_innert = scratch.tile([R, 1], F32, tag="ti2")
                nc.vector.tensor_copy(sb_innert[:], innert_ps[:])
                yield

                ab_ps = psum_t.tile([1, 2 * CO], F32, tag="tiny")
                nc.tensor.matmul(
                    ab_ps[:], lhsT=sb_innert[:], rhs=phicat,
                    start=True, stop=True,
                )
                sb_ab = scratch.tile([1, 2 * CO], F32, tag="ti3")
                nc.vector.tensor_copy(sb_ab[:], ab_ps[:])
                yield

                outs = []
                for k in range(3):  # bias_even (A*y_even), bias_odd, scale (B)
                    ps = psum_bc.tile([128, 2 * CO], F32, tag="bc")
                    nc.tensor.matmul(
                        ps[:],
                        lhsT=ybc[0:1, 128 * k : 128 * (k + 1)],
                        rhs=sb_ab[:],
                        start=True,
                        stop=True,
                    )
                    sb = bc_pool.tile([128, 2 * CO], F32, tag="bcs")
                    nc.vector.tensor_copy(sb[:], ps[:])
                    outs.append(sb)
                out["bc"] = outs  # [bias_even, bias_odd, scale]
                yield

            def stage_c_gen(b, bias_e, bias_o, scale):
                eng = 0
                for oc in range(NOB):
                    yield
                    ot = out_pool.tile([128, OBLK, 2, W], BF16, tag="out")
                    for ol in range(OBLK):
                        o = oc * OBLK + ol
                        sc_ap = scale[:, 2 * o + 1 : 2 * o + 2]
                        for hh in range(2):
                            bias_ap = (bias_e if hh == 0 else bias_o)[
                                :, 2 * o : 2 * o + 1
                            ]
                            dst = ot[:, ol, hh, :]
                            which = _GEN_ENGINES[eng % len(_GEN_ENGINES)]
                            eng += 1
                            if which == "dve":
                                nc.vector.tensor_scalar(
                                    out=dst, in0=xrep, scalar1=sc_ap,
                                    scalar2=bias_ap, op0=MULT, op1=ADD,
                                )
                            elif which == "pool":
                                nc.gpsimd.tensor_scalar(
                                    out=dst, in0=xrep, scalar1=sc_ap,
                                    scalar2=bias_ap, op0=MULT, op1=ADD,
                                )
                            else:
                                nc.scalar.activation(
                                    dst, xrep,
                                    mybir.ActivationFunctionType.Identity,
                                    bias=bias_ap, scale=sc_ap,
                                )
                    nc.scalar.dma_start(
                        u5[b, :, oc * OBLK : (oc + 1) * OBLK, :, :],
                        ot[:],
                    )

            stage_a(0)

            state = {}

            def inter0():
                # Pumped twice per A1 chunk (8 pumps over 4 chunks): tiny(0)
                # lands in 3 pieces, then gen-0 block 0; remaining gen-0
                # blocks are emitted after the loop.
                t0 = {}
                state["t0"] = t0
                yield from tiny(0, t0)
                state["g"] = stage_c_gen(0, *t0["bc"])
                next(state["g"], None)
                next(state["g"], None)  # block 0
                yield

            stage_a(1, interleave=inter0)
            for _ in state["g"]:        # gen-0 blocks 1..
                pass
            t1 = {}
            for _ in tiny(1, t1):
                pass
            for _ in stage_c_gen(1, *t1["bc"]):
                pass

    nc.compile()
    return nc


def make_in_maps(v, psi, phi):
    import ml_dtypes
    bf16 = ml_dtypes.bfloat16
    y = np.linspace(-1.0, 1.0, H, dtype=np.float32)
    x = np.linspace(-1.0, 1.0, W, dtype=np.float32)
    dx = np.float32(2.0 / (W - 1))
    dy = np.float32(2.0 / (H - 1))

    cf32 = np.zeros((128, CF32_W), np.float32)
    # wty: row 2i = 1 (y_even-weighted sums); row 2i+1 cols [W:2W) = dy
    cf32[0::2, _WTY : _WTY + 2 * W] = 1.0
    cf32[1::2, _WTY + W : _WTY + 2 * W] = dy
    # wtx: row 2i+1 = x (both hh halves)
    cf32[1::2, _WTX : _WTX + W] = x
    cf32[1::2, _WTX + W : _WTX + 2 * W] = x
    # psi packs (dx folded in)
    cf32[0::2, _PSIY : _PSIY + R] = psi[:, :, 0].T * dx
    cf32[1::2, _PSIY : _PSIY + R] = psi[:, :, 0].T * dx
    cf32[1::2, _PSIX : _PSIX + R] = psi[:, :, 1].T * dx
    # phicat[r, 2o+c] = phi[o, r, c]
    cf32[0:R, _PHI : _PHI + 2 * CO] = np.stack(
        [phi[:, :, 0].T, phi[:, :, 1].T], axis=2
    ).reshape(R, 2 * CO)
    cf32[0, _YBC : _YBC + 128] = y[0::2]
    cf32[0, _YBC + 128 : _YBC + 256] = y[1::2]
    cf32[0, _YBC + 256 : _YBC + 384] = 1.0
    cf32[0, _ID1] = 1.0

    cf16 = np.zeros((128, CBF16_W), np.float32)
    cf16[:, _YLHS + 128] = y[0::2]
    cf16[:, _YLHS + 129] = 1.0
    cf16[:, _XREP : _XREP + W] = x
    cf16 = cf16.astype(bf16)

    # v[b, i, h, w] -> [b, p, i, hh, w] bf16
    v16 = v.astype(bf16)
    vt = v16.reshape(B, CI, HP, 2, W).transpose(0, 2, 1, 3, 4)

    common = {"cf32": cf32, "cf16": cf16}
    return [
        {
            "v5": np.ascontiguousarray(vt[BPC * c : BPC * (c + 1)]),
            **common,
        }
        for c in range(N_CORES)
    ]


def gather_out(results):
    """Per-core u5 [BPC, HP, CO, 2, W] bf16 -> full u [B, CO, H, W] f32."""
    arr = np.stack([r["u5"] for r in results])  # [8, BPC, HP, CO, 2, W]
    arr = arr.transpose(0, 1, 3, 2, 4, 5)       # [8, BPC, CO, HP, 2, W]
    return np.ascontiguousarray(
        arr.reshape(B, CO, H, W).astype(np.float32)
    )


_NC_CACHE = None


def kernel(v, psi, phi):
    global _NC_CACHE
    if _NC_CACHE is None:
        _NC_CACHE = build_nc()
    nc = _NC_CACHE
    in_maps = make_in_maps(
        np.asarray(v, dtype=np.float32),
        np.asarray(psi, dtype=np.float32),
        np.asarray(phi, dtype=np.float32),
    )
    res = run_bass_kernel_spmd(nc, in_maps, core_ids=list(range(N_CORES)))
    return gather_out(res.results)


if __name__ == "__main__":
    build_nc()
    print("build ok")


# revision 38
# speedup vs baseline: 1.0257x; 1.0257x over previous
"""Trainium2 Bass kernel for the low-rank linear operator.

Math: the reference collapses algebraically. With y = linspace(-1,1,H),
x = linspace(-1,1,W), dx = 2/(W-1):

  Vy[b,i] = sum_{h,w} v[b,i,h,w] * y_h
  Vx[b,i] = sum_{h,w} v[b,i,h,w] * x_w
  inner[b,r] = dx * sum_i (Vy[b,i]*psi[r,i,0] + Vx[b,i]*psi[r,i,1])
  A[b,o] = sum_r inner[b,r]*phi[o,r,0];  Bc[b,o] = sum_r inner[b,r]*phi[o,r,1]
  u[b,o,h,w] = A[b,o]*y_h + Bc[b,o]*x_w

Sharding: data-parallel over batch, 2 batches per core, 8 cores, no
collectives.

The problem is HBM-bandwidth bound (read v, write u); bf16 streams halve
DMA traffic vs f32 (measured rel err ~4e-4 against the 2e-2 gate). Host
pre-transposes v to [b, p=h//2, i, hh=h%2, w] bf16 so each input DMA
descriptor is 16KB contiguous; u is produced in the mirrored layout (8KB
descriptors) and re-transposed on the host.

Reduction: partition p = h//2. For each channel ch one matmul with a
sliding-window lhsT (zeros except col 2ch -> y_even values, col 2ch+1 ->
ones) over rhs [128, (hh,w)=512] accumulates, for ALL 64 channels, the
y_even-weighted row sums (psum row 2ch) and plain column sums (row
2ch+1) into a single [128, 512] f32 psum bank. Full-width DVE
mult+reduce passes against wty (1; dy on the hh=1 half) and wtx (0; x)
then give the gy/gx vectors feeding tiny f32 matmuls:
inner -> (A,B) -> per-partition scale/bias tiles. DVE/ACT/Pool
tensor_scalar ops generate u tiles as x_w*B + (y_even|y_odd)*A.

All constants ride in two packed tensors (one f32, one bf16) loaded on
the scalar DMA ring so the first v read issues immediately on sync.
"""

import sys

try:
    import concourse.bass as bass  # noqa: F401
except ImportError:
    for _p in ("/opt/trn_rl_repo", "/root/.axon_site/_ro/trn_rl_repo"):
        if _p not in sys.path:
            sys.path.insert(0, _p)

import numpy as np

import concourse.bacc as bacc
import concourse.bass as bass
import concourse.mybir as mybir
import concourse.tile as tile
from concourse.bass_utils import run_bass_kernel_spmd

F32 = mybir.dt.float32
BF16 = mybir.dt.bfloat16
MULT = mybir.AluOpType.mult
ADD = mybir.AluOpType.add

B, CI, CO, R, H, W = 16, 64, 64, 64, 256, 256
N_CORES = 8
BPC = B // N_CORES  # batches per core
HP = H // 2         # h-pairs per partition dim

IBLK = 16           # input channels per DMA (2MB bf16, 16KB descriptors)
NIB = CI // IBLK
OBLK = 8            # output channels per DMA (1MB bf16, 8KB descriptors)
NOB = CO // OBLK

# packed-constant column offsets (cf32 [128, CF32_W] f32)
_WTY = 0            # [128, 512]
_WTX = 512          # [128, 512]
_PSIY = 1024        # [128, 64]
_PSIX = 1088        # [128, 64]
_PHI = 1152         # [64, 128]
_YBC = 1280         # [1, 384]: y_even | y_odd | ones
_ID1 = 1664         # [1, 1]
CF32_W = 1665
# cf16 [128, CBF16_W] bf16: sliding-window lhsT table then x replicated
_YLHS = 0           # [128, 384]: col 128 = y_even, col 129 = ones
_XREP = 384         # [128, 256]
CBF16_W = 640

# generation-engine rotation, rates ~ DVE 354ns / ACT 585ns / Pool 655ns
_GEN_ENGINES = (
    "dve", "act", "pool", "dve", "dve", "act", "pool", "dve",
    "act", "pool", "dve", "dve", "act", "pool", "dve",
)


def build_nc():
    nc = bacc.Bacc("TRN2", target_bir_lowering=False, debug=False)

    v5 = nc.dram_tensor("v5", [BPC, HP, CI, 2, W], BF16, kind="ExternalInput")
    cf32d = nc.dram_tensor("cf32", [128, CF32_W], F32, kind="ExternalInput")
    cf16d = nc.dram_tensor("cf16", [128, CBF16_W], BF16, kind="ExternalInput")
    u5 = nc.dram_tensor("u5", [BPC, HP, CO, 2, W], BF16, kind="ExternalOutput")

    with tile.TileContext(nc) as tc:
        with (
            tc.tile_pool(name="consts", bufs=1) as consts,
            tc.tile_pool(name="inp", bufs=4) as in_pool,
            tc.tile_pool(name="outp", bufs=5) as out_pool,
            tc.tile_pool(name="scr", bufs=3) as scratch,
            tc.tile_pool(name="bc", bufs=6) as bc_pool,
            tc.tile_pool(name="psumA", bufs=2, space="PSUM") as psum_a,
            tc.tile_pool(name="psumT", bufs=1, space="PSUM") as psum_t,
            tc.tile_pool(name="psumBC", bufs=3, space="PSUM") as psum_bc,
        ):
            # cf16 gates the first matmul: tiny, lands first on the sync ring
            # ahead of the v reads. cf32 is only needed by stage-2 (~30us in);
            # the gpsimd ring keeps it off both streaming rings.
            cf16 = consts.tile([128, CBF16_W], BF16)
            nc.sync.dma_start(cf16[:], cf16d[:])
            cf32 = consts.tile([128, CF32_W], F32)
            nc.scalar.dma_start(cf32[:], cf32d[:])

            wty = cf32[:, _WTY : _WTY + 2 * W]
            wtx = cf32[:, _WTX : _WTX + 2 * W]
            psi2y = cf32[:, _PSIY : _PSIY + R]
            psi2x = cf32[:, _PSIX : _PSIX + R]
            phicat = cf32[0:R, _PHI : _PHI + 2 * CO]
            ybc = cf32[0:1, _YBC : _YBC + 384]
            id1 = cf32[0:1, _ID1 : _ID1 + 1]
            xrep = cf16[:, _XREP : _XREP + W]

            # per-batch reduction vectors, one column per batch
            gy_sb = consts.tile([2 * CI, BPC], F32)
            gx_sb = consts.tile([2 * CI, BPC], F32)

            def stage_a(b, interleave=None):
                """Reduce v[b] -> gy_sb/gx_sb[:, b]."""
                inter = interleave() if interleave is not None else None
                ps = psum_a.tile([128, 2, W], F32, tag="A")
                for blk in range(NIB):
                    t = in_pool.tile([128, IBLK, 2, W], BF16, tag="in")
                    nc.sync.dma_start(
                        t[:],
                        v5[b, :, blk * IBLK : (blk + 1) * IBLK, :, :],
                    )
                    for ii in range(IBLK):
                        ch = blk * IBLK + ii
                        lo = _YLHS + 128 - 2 * ch
                        nc.tensor.matmul(
                            ps[:],
                            lhsT=cf16[:, lo : lo + 128],
                            rhs=t[:, ii, :, :],
                            start=(ch == 0),
                            stop=(ch == CI - 1),
                        )
                    if inter is not None:
                        next(inter, None)
                        next(inter, None)
                psv = ps[:].rearrange("p hh w -> p (hh w)")
                sc = scratch.tile([128, 2 * W], F32, tag="sc")
                nc.vector.tensor_tensor(out=sc[:], in0=psv, in1=wty, op=MULT)
                nc.vector.tensor_reduce(
                    out=gy_sb[:, b : b + 1], in_=sc[:],
                    axis=mybir.AxisListType.X, op=ADD,
                )
                sc2 = scratch.tile([128, 2 * W], F32, tag="sc")
                nc.vector.tensor_tensor(out=sc2[:], in0=psv, in1=wtx, op=MULT)
                nc.vector.tensor_reduce(
                    out=gx_sb[:, b : b + 1], in_=sc2[:],
                    axis=mybir.AxisListType.X, op=ADD,
                )

            def tiny(b, out):
                """gy/gx[:, b] -> scale/bias tiles; emitted in 3 pumps so
                each PE hop's DVE-side input exists before the PE reaches it
                (no PE-queue stall between batch-1 matmul chunks)."""
                innert_ps = psum_t.tile([R, 1], F32, tag="tiny")
                nc.tensor.matmul(
                    innert_ps[:], lhsT=psi2y, rhs=gy_sb[:, b : b + 1],
                    start=True, stop=False,
                )
                nc.tensor.matmul(
                    innert_ps[:], lhsT=psi2x, rhs=gx_sb[:, b : b + 1],
                    start=False, stop=True,
                )
                sb_innert = scratch.tile([R, 1], F32, tag="ti2")
                nc.vector.tensor_copy(sb_innert[:], innert_ps[:])
                yield

                ab_ps = psum_t.tile([1, 2 * CO], F32, tag="tiny")
                nc.tensor.matmul(
                    ab_ps[:], lhsT=sb_innert[:], rhs=phicat,
                    start=True, stop=True,
                )
                sb_ab = scratch.tile([1, 2 * CO], F32, tag="ti3")
                nc.vector.tensor_copy(sb_ab[:], ab_ps[:])
                yield

                outs = []
                for k in range(3):  # bias_even (A*y_even), bias_odd, scale (B)
                    ps = psum_bc.tile([128, 2 * CO], F32, tag="bc")
                    nc.tensor.matmul(
                        ps[:],
                        lhsT=ybc[0:1, 128 * k : 128 * (k + 1)],
                        rhs=sb_ab[:],
                        start=True,
                        stop=True,
                    )
                    sb = bc_pool.tile([128, 2 * CO], F32, tag="bcs")
                    nc.vector.tensor_copy(sb[:], ps[:])
                    outs.append(sb)
                out["bc"] = outs  # [bias_even, bias_odd, scale]
                yield

            def stage_c_gen(b, bias_e, bias_o, scale):
                eng = 0
                for oc in range(NOB):
                    yield
                    ot = out_pool.tile([128, OBLK, 2, W], BF16, tag="out")
                    for ol in range(OBLK):
                        o = oc * OBLK + ol
                        sc_ap = scale[:, 2 * o + 1 : 2 * o + 2]
                        for hh in range(2):
                            bias_ap = (bias_e if hh == 0 else bias_o)[
                                :, 2 * o : 2 * o + 1
                            ]
                            dst = ot[:, ol, hh, :]
                            which = _GEN_ENGINES[eng % len(_GEN_ENGINES)]
                            eng += 1
                            if which == "dve":
                                nc.vector.tensor_scalar(
                                    out=dst, in0=xrep, scalar1=sc_ap,
                                    scalar2=bias_ap, op0=MULT, op1=ADD,
                                )
                            elif which == "pool":
                                nc.gpsimd.tensor_scalar(
                                    out=dst, in0=xrep, scalar1=sc_ap,
                                    scalar2=bias_ap, op0=MULT, op1=ADD,
                                )
                            else:
                                nc.scalar.activation(
                                    dst, xrep,
                                    mybir.ActivationFunctionType.Identity,
                                    bias=bias_ap, scale=sc_ap,
                                )
                    nc.scalar.dma_start(
                        u5[b, :, oc * OBLK : (oc + 1) * OBLK, :, :],
                        ot[:],
                    )

            stage_a(0)

            state = {}

            def inter0():
                # Pumped twice per A1 chunk (8 pumps over 4 chunks): tiny(0)
                # lands in 3 pieces, then gen-0 block 0; remaining gen-0
                # blocks are emitted after the loop.
                t0 = {}
                state["t0"] = t0
                yield from tiny(0, t0)
                state["g"] = stage_c_gen(0, *t0["bc"])
                next(state["g"], None)
                next(state["g"], None)  # block 0
                yield

            stage_a(1, interleave=inter0)
            for _ in state["g"]:        # gen-0 blocks 1..
                pass
            t1 = {}
            for _ in tiny(1, t1):
                pass
            for _ in stage_c_gen(1, *t1["bc"]):
                pass

    nc.compile()
    return nc


def make_in_maps(v, psi, phi):
    import ml_dtypes
    bf16 = ml_dtypes.bfloat16
    y = np.linspace(-1.0, 1.0, H, dtype=np.float32)
    x = np.linspace(-1.0, 1.0, W, dtype=np.float32)
    dx = np.float32(2.0 / (W - 1))
    dy = np.float32(2.0 / (H - 1))

    cf32 = np.zeros((128, CF32_W), np.float32)
    # wty: row 2i = 1 (y_even-weighted sums); row 2i+1 cols [W:2W) = dy
    cf32[0::2, _WTY : _WTY + 2 * W] = 1.0
    cf32[1::2, _WTY + W : _WTY + 2 * W] = dy
    # wtx: row 2i+1 = x (both hh halves)
    cf32[1::2, _WTX : _WTX + W] = x
    cf32[1::2, _WTX + W : _WTX + 2 * W] = x
    # psi packs (dx folded in)
    cf32[0::2, _PSIY : _PSIY + R] = psi[:, :, 0].T * dx
    cf32[1::2, _PSIY : _PSIY + R] = psi[:, :, 0].T * dx
    cf32[1::2, _PSIX : _PSIX + R] = psi[:, :, 1].T * dx
    # phicat[r, 2o+c] = phi[o, r, c]
    cf32[0:R, _PHI : _PHI + 2 * CO] = np.stack(
        [phi[:, :, 0].T, phi[:, :, 1].T], axis=2
    ).reshape(R, 2 * CO)
    cf32[0, _YBC : _YBC + 128] = y[0::2]
    cf32[0, _YBC + 128 : _YBC + 256] = y[1::2]
    cf32[0, _YBC + 256 : _YBC + 384] = 1.0
    cf32[0, _ID1] = 1.0

    cf16 = np.zeros((128, CBF16_W), np.float32)
    cf16[:, _YLHS + 128] = y[0::2]
    cf16[:, _YLHS + 129] = 1.0
    cf16[:, _XREP : _XREP + W] = x
    cf16 = cf16.astype(bf16)

    # v[b, i, h, w] -> [b, p, i, hh, w] bf16
    v16 = v.astype(bf16)
    vt = v16.reshape(B, CI, HP, 2, W).transpose(0, 2, 1, 3, 4)

    common = {"cf32": cf32, "cf16": cf16}
    return [
        {
            "v5": np.ascontiguousarray(vt[BPC * c : BPC * (c + 1)]),
            **common,
        }
        for c in range(N_CORES)
    ]


def gather_out(results):
    """Per-core u5 [BPC, HP, CO, 2, W] bf16 -> full u [B, CO, H, W] f32."""
    arr = np.stack([r["u5"] for r in results])  # [8, BPC, HP, CO, 2, W]
    arr = arr.transpose(0, 1, 3, 2, 4, 5)       # [8, BPC, CO, HP, 2, W]
    return np.ascontiguousarray(
        arr.reshape(B, CO, H, W).astype(np.float32)
    )


_NC_CACHE = None


def kernel(v, psi, phi):
    global _NC_CACHE
    if _NC_CACHE is None:
        _NC_CACHE = build_nc()
    nc = _NC_CACHE
    in_maps = make_in_maps(
        np.asarray(v, dtype=np.float32),
        np.asarray(psi, dtype=np.float32),
        np.asarray(phi, dtype=np.float32),
    )
    res = run_bass_kernel_spmd(nc, in_maps, core_ids=list(range(N_CORES)))
    return gather_out(res.results)


if __name__ == "__main__":
    build_nc()
    print("build ok")
